# revision 17
# baseline (speedup 1.0000x reference)
"""Trainium2 Bass kernel for dense-MoE routing (8 experts, gate-weighted sum).

Math (restructured from the reference):
    gate   = softmax(x @ wg + bg)                  per token, E=8
    h      = relu(x @ W1cat + b1cat)               W1cat = w1 transposed/concat [C, E*H]
    out    = (gate-scaled h) @ W2p + gate @ B2W    W2p = w2.reshape(E*H,EO) @ wo (host-folded)
                                                   B2W = b2 @ wo + bo  (sum(gate)=1 absorbs bo)

Sharding: data-parallel over tokens; core i takes batch row i (4096 tokens).
All weights replicated.  Everything on-chip is token-moving (features on
partitions).  mm1 + gate run in fp32r (11-bit mantissa, full PE rate at
N>=256); mm2 runs in bf16 (hs produced in bf16, FWL weight loads).
"""

import numpy as np

_P = 128          # partitions
_T = 4096         # tokens per core
_TN = 512         # token chunk (matmul moving dim)
_NTN = _T // _TN  # 8
_HM = 32          # hid tiles (4096 / 128)
_E = 8
_OC = 256         # output channels
_NCORES = 8

_CACHE = {}


def _round_fp32r(a):
    """Round fp32 to fp32r (1s/8e/11m, low 12 bits zero), round-to-nearest-even."""
    u = np.ascontiguousarray(a, np.float32).view(np.uint32)
    low = u & np.uint32(0xFFF)
    base = u & np.uint32(0xFFFFF000)
    lsb = (u >> np.uint32(12)) & np.uint32(1)
    add = (low > 0x800) | ((low == 0x800) & (lsb == 1))
    return (base + (add.astype(np.uint32) << np.uint32(12))).view(np.float32)


def _build_nc(reps=1, loop=1, stagger=False, gbdma=True, fine=True, obact=True, chunkload=True, podma=False, dverelu=True, ph5=False, tn0split=True, pp=False, po3=False, ppk=0, mm2t=False, mm1bf=True, nogmul=False, allact=False, poolmul=0, bgact=True):
    import concourse.bacc as bacc
    import concourse.bass as bass
    import concourse.mybir as mybir
    import concourse.tile as tile

    f32 = mybir.dt.float32
    f32r = mybir.dt.float32r
    bf16 = mybir.dt.bfloat16
    AF = mybir.ActivationFunctionType
    ts = bass.ts

    nc = bacc.Bacc("TRN2", target_bir_lowering=False, debug=False)

    # float32r tensors carry host-pre-rounded fp32r bit patterns (low 12 bits
    # zero), so the DMA chain stays type-consistent for the BIR verifier.
    # mm1bf: x/w1/gate path in bf16 (FWL weight loads, half the DMA bytes)
    xdt = bf16 if mm1bf else f32r
    xT_d = nc.dram_tensor("xT", [2, _NTN, _P, _TN], xdt, kind="ExternalInput").ap()
    w1_d = nc.dram_tensor("w1s", [2, 4, _P, 1024], xdt, kind="ExternalInput").ap()
    w2p_d = nc.dram_tensor("w2ps", [_P, _HM, _OC], bf16, kind="ExternalInput").ap()
    b1_d = nc.dram_tensor("b1s", [_P, _HM], f32, kind="ExternalInput").ap()
    wg_d = nc.dram_tensor("wgs", [2, _P, _E], xdt, kind="ExternalInput").ap()
    bg_d = nc.dram_tensor("bgs", [1, _E], xdt, kind="ExternalInput").ap()
    if bgact:
        bgc_d = nc.dram_tensor("bgc", [_E, 1], f32, kind="ExternalInput").ap()
    b2w_d = nc.dram_tensor("b2ws", [_E, _OC], f32r, kind="ExternalInput").ap()
    ones_d = nc.dram_tensor("ones", [_E, _TN], xdt, kind="ExternalInput").ap()
    gst_d = nc.dram_tensor("gstage", [_NTN, _E, _TN], bf16).ap()
    # mm2t: out stored transposed [octile, oc, tok]; host re-lays it out
    if mm2t:
        out_d = nc.dram_tensor("out", [2, _P, _T], f32, kind="ExternalOutput").ap()
    else:
        out_d = nc.dram_tensor("out", [_T, _OC], f32, kind="ExternalOutput").ap()

    def asf32(ap):
        return ap.bitcast(f32)

    import contextlib

    @contextlib.contextmanager
    def _nullpool():
        yield None

    with tile.TileContext(nc) as tc:
        with (
            tc.tile_pool(name="const", bufs=1) as const,
            tc.tile_pool(name="hs", bufs=2) as p_hs,
            tc.tile_pool(name="gb", bufs=2) as p_gb,
            tc.tile_pool(name="gsmall", bufs=2) as p_gs,
            tc.tile_pool(name="gaten", bufs=3) as p_gn,
            tc.tile_pool(name="ob", bufs=4) as p_ob,
            tc.tile_pool(name="psum_h", bufs=5 if ph5 else 4, space="PSUM") as psum_h,
            tc.tile_pool(name="psum_o", bufs=3 if po3 else 2, space="PSUM") as psum_o,
            tc.tile_pool(name="psum_g", bufs=1, space="PSUM") as psum_g,
            tc.tile_pool(name="psum_s", bufs=1, space="PSUM") if not (ph5 or po3) else _nullpool() as psum_s,
        ):
            xT_sb = const.tile([_P, 2, _T], xdt, name="xT_sb")
            w1_sb = const.tile([_P, 2, 4096], xdt, name="w1_sb")
            w2p_sb = const.tile([_P, _HM, _OC], bf16, name="w2p_sb")
            b1_sb = const.tile([_P, _HM], f32, name="b1_sb")
            wg_sb = const.tile([_P, 2, _E], xdt, name="wg_sb")
            bg_sb = const.tile([1, _E], xdt, name="bg_sb")
            bgc_sb = const.tile([_E, 1], f32, name="bgc_sb") if bgact else None
            b2w_sb = const.tile([_E, _OC], f32r, name="b2w_sb")
            ones_sb = const.tile([_E, _TN], xdt, name="ones_sb")

            for kc in range(2):
                nc.sync.dma_start(out=wg_sb[:, kc, :], in_=wg_d[kc])
            # x chunk 0 next: the gate matmul only needs wg + x0
            for kc in range(2):
                nc.sync.dma_start(out=xT_sb[:, kc, ts(0, _TN)], in_=xT_d[kc, 0])
            nc.sync.dma_start(out=bg_sb[:], in_=bg_d[:])
            if bgact:
                nc.sync.dma_start(out=bgc_sb[:], in_=bgc_d[:])
            nc.sync.dma_start(out=ones_sb[:], in_=ones_d[:])
            # b1 (16KB) before w1: the first relu drains gate on it; queued
            # behind w1's 2MB it stalls the chunk-0 psum_h drain ~2us
            nc.sync.dma_start(out=b1_sb[:], in_=b1_d[:])
            # w1 split fine (512KB pieces) so mm1 hm=0 starts early
            for q in range(4):
                for kc in range(2):
                    for hq in range(2):
                        nc.sync.dma_start(out=w1_sb[:, kc, ts(2 * q + hq, 512)],
                                          in_=w1_d[kc, q, :, ts(hq, 512)])
            # w2p/b2w before x1-7: mm2 of chunk 0 (interleaved into chunk 1's
            # mm1, ~t=30us) needs w2p; x chunk tn isn't needed until its own
            # mm1 window (~tn*27us), so it can queue behind the 2MB w2p.
            nc.sync.dma_start(out=b2w_sb[:], in_=b2w_d[:])
            nc.sync.dma_start(out=w2p_sb[:], in_=w2p_d[:])
            for tn in range(1, _NTN):
                for kc in range(2):
                    nc.sync.dma_start(out=xT_sb[:, kc, ts(tn, _TN)], in_=xT_d[kc, tn])

            def emit_gate(tn):
                pg = psum_g.tile([_E, _TN], f32, name="pg", tag="pg")
                nc.tensor.matmul(pg[:], wg_sb[:, 0, :], xT_sb[:, 0, ts(tn, _TN)],
                                 start=True, stop=False)
                if bgact:
                    nc.tensor.matmul(pg[:], wg_sb[:, 1, :], xT_sb[:, 1, ts(tn, _TN)],
                                     start=False, stop=True)
                else:
                    nc.tensor.matmul(pg[:], wg_sb[:, 1, :], xT_sb[:, 1, ts(tn, _TN)],
                                     start=False, stop=False)
                    nc.tensor.matmul(pg[:], bg_sb[:], ones_sb[0:1, :],
                                     start=False, stop=True)
                expu = p_gs.tile([_E, _TN], xdt, name="expu", tag="expu")
                if bgact:
                    nc.scalar.activation(expu[:], pg[:], AF.Exp, bias=bgc_sb[:, 0:1])
                else:
                    nc.scalar.activation(expu[:], pg[:], AF.Exp)
                if ph5 or po3:
                    ps = psum_g.tile([_E, _TN], f32, name="ps", tag="pg")[0:1, :]
                else:
                    ps = psum_s.tile([1, _TN], f32, name="ps", tag="ps")
                nc.tensor.matmul(ps[:], ones_sb[:, 0:1], expu[:],
                                 start=True, stop=True)
                rc = p_gs.tile([1, _TN], f32, name="rc", tag="rc")
                nc.vector.reciprocal(rc[:], ps[:])
                rcb = p_gs.tile([_E, _TN], f32, name="rcb", tag="rcb")
                nc.gpsimd.partition_broadcast(rcb[:], rc[:])
                gaten = p_gn.tile([_E, _TN], f32r, name="gaten", tag="gaten")
                nc.vector.tensor_mul(gaten[:], expu[:] if mm1bf else asf32(expu[:]),
                                     rcb[:])
                gatenb = p_gs.tile([_E, _TN], bf16, name="gatenb", tag="gatenb")
                nc.vector.tensor_copy(gatenb[:], asf32(gaten[:]))
                # gpsimd partition_broadcast needs base partition 0: DMA the 8
                # gate rows onto partition 0 of gb, then broadcast in place.
                gb = p_gb.tile([_P, _E, _TN], bf16, name="gb", tag="gb")
                if gbdma:
                    nc.sync.dma_start(out=gst_d[tn], in_=gatenb[:])
                    for e in range(_E):
                        src_bc = gst_d[tn, e:e + 1, :].broadcast_to((_P, _TN))
                        nc.sync.dma_start(out=gb[:, e, :], in_=src_bc)
                else:
                    nc.sync.dma_start(out=gb[0:1, :, :], in_=gatenb[:])
                    for e in range(_E):
                        nc.gpsimd.partition_broadcast(gb[:, e, :], gb[0:1, e, :])
                return gaten, gb

            def emit_mm1_pair(tn, hm, hs, gb):
                ph = psum_h.tile([_P, _TN], f32, name="ph", tag="ph")
                nc.tensor.matmul(ph[:], w1_sb[:, 0, ts(hm, _P)],
                                 xT_sb[:, 0, ts(tn, _TN)], start=True, stop=False)
                nc.tensor.matmul(ph[:], w1_sb[:, 1, ts(hm, _P)],
                                 xT_sb[:, 1, ts(tn, _TN)], start=False, stop=True)
                # relu(+bias) straight into bf16 hs, then scale by gate in place.
                # tn==0 has no previous-chunk mm2 to interleave, so the PE is
                # paced by the psum drain there: split that drain ACT/DVE (the
                # DVE is otherwise idle during tn==0's mm1).
                if not allact and tn0split and tn == 0 and hm % 2 == 1:
                    nc.vector.tensor_scalar(hs[:, hm, :], ph[:],
                                            b1_sb[:, hm:hm + 1], 0.0,
                                            mybir.AluOpType.add,
                                            mybir.AluOpType.max)
                elif not allact and dverelu and hm % 4 == 3:
                    nc.vector.tensor_scalar(hs[:, hm, :], ph[:],
                                            b1_sb[:, hm:hm + 1], 0.0,
                                            mybir.AluOpType.add,
                                            mybir.AluOpType.max)
                else:
                    nc.scalar.activation(hs[:, hm, :], ph[:], AF.Relu,
                                         bias=b1_sb[:, hm:hm + 1])
                if not nogmul:
                    eng = nc.gpsimd if (hm % 4) < poolmul else nc.vector
                    eng.tensor_mul(hs[:, hm, :], hs[:, hm, :], gb[:, hm // 4, :])

            po_open = {}

            def emit_mm2_half(tn, hs, gaten, tw, half):
                if half == 0:
                    po_open[tw] = psum_o.tile([_P, _OC], f32, name="po", tag="po")
                    for kt in range(_HM // 2):
                        nc.tensor.matmul(po_open[tw][:], hs[:, kt, ts(tw, _P)],
                                         w2p_sb[:, kt, :], start=(kt == 0), stop=False)
                    return
                po = po_open.pop(tw)
                for kt in range(_HM // 2, _HM):
                    nc.tensor.matmul(po[:], hs[:, kt, ts(tw, _P)], w2p_sb[:, kt, :],
                                     start=False, stop=False)
                nc.tensor.matmul(po[:], gaten[:, ts(tw, _P)], b2w_sb[:],
                                 start=False, stop=True)
                finish_mm2(tn, po, tw)

            def emit_mm2_block(tn, hs, gaten, tw):
                po = psum_o.tile([_P, _OC], f32, name="po", tag="po")
                for kt in range(_HM):
                    nc.tensor.matmul(po[:], hs[:, kt, ts(tw, _P)], w2p_sb[:, kt, :],
                                     start=(kt == 0), stop=False)
                nc.tensor.matmul(po[:], gaten[:, ts(tw, _P)], b2w_sb[:],
                                 start=False, stop=True)
                finish_mm2(tn, po, tw)

            def finish_mm2(tn, po, tw):
                row = (tn * (_TN // _P) + tw) * _P
                if podma:
                    nc.sync.dma_start(out=out_d[row:row + _P, :], in_=po[:])
                    return
                ob = p_ob.tile([_P, _OC], f32, name="ob", tag="ob")
                if obact:
                    nc.scalar.copy(ob[:], po[:])
                else:
                    nc.vector.tensor_copy(ob[:], po[:])
                nc.sync.dma_start(out=out_d[row:row + _P, :], in_=ob[:])

            NTW = _TN // _P  # mm2 token windows per chunk (4)
            HM_PER_TW = _HM // NTW  # mm1 pairs between mm2 blocks (8)

            def finish_mm2t(tn, po, octile):
                ob = p_ob.tile([_P, _TN], f32, name="ob", tag="ob")
                nc.scalar.copy(ob[:], po[:])
                nc.sync.dma_start(out=out_d[octile, :, ts(tn, _TN)], in_=ob[:])

            # mm2t quarter-chains for the fine interleave: octile chain of 32
            # kt + b2w split into 4 pieces, psum group held open across them
            po_t_open = {}

            def emit_mm2t_quarter(tn, hs, gaten, octile, q):
                if q == 0:
                    po_t_open[octile] = psum_o.tile([_P, _TN], f32, name="po",
                                                    tag="po")
                po = po_t_open[octile]
                for kt in range(q * (_HM // 4), (q + 1) * (_HM // 4)):
                    nc.tensor.matmul(po[:], w2p_sb[:, kt, ts(octile, _P)],
                                     hs[:, kt, :], start=(kt == 0), stop=False)
                if q == 3:
                    nc.tensor.matmul(po[:], b2w_sb[:, ts(octile, _P)], gaten[:],
                                     start=False, stop=True)
                    finish_mm2t(tn, po_t_open.pop(octile), octile)

            def mm2_steps(tn, hs, gaten):
                """Yield single mm2 PE ops for one chunk."""
                if mm2t:
                    # transposed: out[oc, tok] chains over kt with w2p
                    # stationary -> 2 chains of 33 N=512 matmuls per chunk
                    # (half the PE instructions of the tw-major form)
                    for octile in range(2):
                        po = psum_o.tile([_P, _TN], f32, name="po", tag="po")
                        for kt in range(_HM):
                            nc.tensor.matmul(po[:], w2p_sb[:, kt, ts(octile, _P)],
                                             hs[:, kt, :],
                                             start=(kt == 0), stop=False)
                            yield
                        nc.tensor.matmul(po[:], b2w_sb[:, ts(octile, _P)], gaten[:],
                                         start=False, stop=True)
                        finish_mm2t(tn, po, octile)
                        yield
                    return
                for tw in range(NTW):
                    po = psum_o.tile([_P, _OC], f32, name="po", tag="po")
                    for kt in range(_HM):
                        nc.tensor.matmul(po[:], hs[:, kt, ts(tw, _P)],
                                         w2p_sb[:, kt, :],
                                         start=(kt == 0), stop=False)
                        yield
                    nc.tensor.matmul(po[:], gaten[:, ts(tw, _P)], b2w_sb[:],
                                     start=False, stop=True)
                    finish_mm2(tn, po, tw)
                    yield

            def emit_body():
                pending = None
                for tn in range(_NTN):
                    gaten, gb = emit_gate(tn)
                    hs = p_hs.tile([_P, _HM, _TN], bf16, name="hs", tag="hs")
                    for hm in range(_HM):
                        emit_mm1_pair(tn, hm, hs, gb)
                        # interleave previous chunk's mm2 into the PE stream so
                        # the PE never stalls on the ACT-gated psum_h drain
                        if pending is None:
                            continue
                        if pp:
                            if mm2t:
                                n = ppk if ppk else 2  # 66 steps / 32 hm
                            else:
                                n = ppk if ppk else (5 if hm % 2 else 4)  # 136 / 32
                            for _ in range(n):
                                if next(pending[0], None) is None:
                                    break
                        elif fine:
                            if hm % 4 == 3:
                                if mm2t:
                                    idx = hm // 4
                                    emit_mm2t_quarter(*pending[1],
                                                      octile=idx // 4, q=idx % 4)
                                else:
                                    emit_mm2_half(*pending[1], tw=hm // 8, half=(hm // 4) % 2)
                        elif hm % HM_PER_TW == HM_PER_TW - 1:
                            emit_mm2_block(*pending[1], tw=hm // HM_PER_TW)
                    if pending is not None and pp:
                        for _ in pending[0]:
                            pass
                    pending = (mm2_steps(tn, hs, gaten), (tn, hs, gaten))
                if pp:
                    for _ in pending[0]:
                        pass
                elif mm2t:
                    for octile in range(2):
                        for q in range(4):
                            emit_mm2t_quarter(*pending[1], octile=octile, q=q)
                else:
                    for tw in range(NTW):
                        emit_mm2_block(*pending[1], tw=tw)

            if loop > 1:
                with tc.For_i(0, loop, 1, staggered_reset=stagger):
                    emit_body()
            else:
                for _rep in range(reps):
                    emit_body()

    nc.compile()
    return nc


_MM1BF = True


def _xcast(a):
    import ml_dtypes
    if _MM1BF:
        return np.asarray(a, np.float32).astype(ml_dtypes.bfloat16)
    return _round_fp32r(a)


def _prep_weights(w1, b1, w2, b2, wg, bg, wo, bo):
    import ml_dtypes
    f32 = np.float32
    w1 = np.asarray(w1, f32)
    w2 = np.asarray(w2, f32)
    wo = np.asarray(wo, f32)
    E, IN, HID = w1.shape
    w1s = _xcast(np.ascontiguousarray(
        w1.transpose(1, 0, 2).reshape(IN, E * HID).reshape(2, _P, 4, 1024)
        .transpose(0, 2, 1, 3)))
    w2p = (w2.astype(np.float64).reshape(E * HID, -1) @ wo.astype(np.float64)).astype(f32)
    w2ps = np.ascontiguousarray(
        w2p.reshape(_HM, _P, _OC).transpose(1, 0, 2)).astype(ml_dtypes.bfloat16)
    b1s = np.ascontiguousarray(np.asarray(b1, f32).reshape(E * HID).reshape(_HM, _P).T)
    b2ws = _round_fp32r((np.asarray(b2, np.float64) @ wo.astype(np.float64)
                         + np.asarray(bo, np.float64)).astype(f32))
    wgs = _xcast(np.ascontiguousarray(np.asarray(wg, f32).reshape(2, _P, E)))
    bgs = _xcast(np.asarray(bg, f32).reshape(1, E))
    ones = np.ones((_E, _TN), f32)
    if _MM1BF:
        ones = ones.astype(ml_dtypes.bfloat16)
    bgc = np.ascontiguousarray(np.asarray(bg, f32).reshape(_E, 1))
    return dict(w1s=w1s, w2ps=w2ps, b1s=b1s, b2ws=b2ws, wgs=wgs, bgs=bgs,
                bgc=bgc, ones=ones)


def make_in_maps(x, w1, b1, w2, b2, wg, bg, wo, bo):
    x = np.asarray(x, np.float32)
    b, n, c = x.shape
    weights = _prep_weights(w1, b1, w2, b2, wg, bg, wo, bo)
    x2d = x.reshape(b * n, c)
    in_maps = []
    for i in range(_NCORES):
        xc = x2d[i * _T:(i + 1) * _T]                       # [T, C]
        xT = _xcast(np.ascontiguousarray(
            xc.T.reshape(2, _P, _NTN, _TN).transpose(0, 2, 1, 3)))
        in_maps.append({"xT": xT, **weights})
    return in_maps


def _run(x, w1, b1, w2, b2, wg, bg, wo, bo, trace=False):
    from concourse.bass_utils import run_bass_kernel_spmd

    if "nc" not in _CACHE:
        _CACHE["nc"] = _build_nc(1)
    nc = _CACHE["nc"]

    x = np.asarray(x, np.float32)
    b, n, c = x.shape
    in_maps = make_in_maps(x, w1, b1, w2, b2, wg, bg, wo, bo)

    res = run_bass_kernel_spmd(nc, in_maps, list(range(_NCORES)), trace=trace)
    def unshard(o):
        if o.ndim == 3:  # mm2t: [octile, oc, tok] -> [tok, oc]
            return o.transpose(2, 0, 1).reshape(_T, _OC)
        return o
    out = np.concatenate(
        [unshard(res.results[i]["out"]) for i in range(_NCORES)], axis=0)
    return out.reshape(b, n, _OC), res


def kernel(x, w1, b1, w2, b2, wg, bg, wo, bo):
    out, _ = _run(x, w1, b1, w2, b2, wg, bg, wo, bo, trace=False)
    return out



# revision 20
# speedup vs baseline: 1.0119x; 1.0119x over previous
"""Trainium2 Bass kernel for dense-MoE routing (8 experts, gate-weighted sum).

Math (restructured from the reference):
    gate   = softmax(x @ wg + bg)                  per token, E=8
    h      = relu(x @ W1cat + b1cat)               W1cat = w1 transposed/concat [C, E*H]
    out    = (gate-scaled h) @ W2p + gate @ B2W    W2p = w2.reshape(E*H,EO) @ wo (host-folded)
                                                   B2W = b2 @ wo + bo  (sum(gate)=1 absorbs bo)

Sharding: data-parallel over tokens; core i takes batch row i (4096 tokens).
All weights replicated.  Everything on-chip is token-moving (features on
partitions).  mm1 + gate run in fp32r (11-bit mantissa, full PE rate at
N>=256); mm2 runs in bf16 (hs produced in bf16, FWL weight loads).
"""

import numpy as np

_P = 128          # partitions
_T = 4096         # tokens per core
_TN = 512         # token chunk (matmul moving dim)
_NTN = _T // _TN  # 8
_HM = 32          # hid tiles (4096 / 128)
_E = 8
_OC = 256         # output channels
_NCORES = 8

_CACHE = {}


def _round_fp32r(a):
    """Round fp32 to fp32r (1s/8e/11m, low 12 bits zero), round-to-nearest-even."""
    u = np.ascontiguousarray(a, np.float32).view(np.uint32)
    low = u & np.uint32(0xFFF)
    base = u & np.uint32(0xFFFFF000)
    lsb = (u >> np.uint32(12)) & np.uint32(1)
    add = (low > 0x800) | ((low == 0x800) & (lsb == 1))
    return (base + (add.astype(np.uint32) << np.uint32(12))).view(np.float32)


def _build_nc(reps=1, loop=1, stagger=False, gbdma=True, fine="q", obact=True, chunkload=True, podma=False, dverelu=True, ph5=False, tn0split=True, pp=False, po3=False, ppk=0, mm2t=False, mm1bf=True, nogmul=False, allact=False, poolmul=0, bgact=True):
    import concourse.bacc as bacc
    import concourse.bass as bass
    import concourse.mybir as mybir
    import concourse.tile as tile

    f32 = mybir.dt.float32
    f32r = mybir.dt.float32r
    bf16 = mybir.dt.bfloat16
    AF = mybir.ActivationFunctionType
    ts = bass.ts

    nc = bacc.Bacc("TRN2", target_bir_lowering=False, debug=False)

    # float32r tensors carry host-pre-rounded fp32r bit patterns (low 12 bits
    # zero), so the DMA chain stays type-consistent for the BIR verifier.
    # mm1bf: x/w1/gate path in bf16 (FWL weight loads, half the DMA bytes)
    xdt = bf16 if mm1bf else f32r
    xT_d = nc.dram_tensor("xT", [2, _NTN, _P, _TN], xdt, kind="ExternalInput").ap()
    w1_d = nc.dram_tensor("w1s", [2, 4, _P, 1024], xdt, kind="ExternalInput").ap()
    w2p_d = nc.dram_tensor("w2ps", [_P, _HM, _OC], bf16, kind="ExternalInput").ap()
    b1_d = nc.dram_tensor("b1s", [_P, _HM], f32, kind="ExternalInput").ap()
    wg_d = nc.dram_tensor("wgs", [2, _P, _E], xdt, kind="ExternalInput").ap()
    bg_d = nc.dram_tensor("bgs", [1, _E], xdt, kind="ExternalInput").ap()
    if bgact:
        bgc_d = nc.dram_tensor("bgc", [_E, 1], f32, kind="ExternalInput").ap()
    b2w_d = nc.dram_tensor("b2ws", [_E, _OC], f32r, kind="ExternalInput").ap()
    ones_d = nc.dram_tensor("ones", [_E, _TN], xdt, kind="ExternalInput").ap()
    gst_d = nc.dram_tensor("gstage", [_NTN, _E, _TN], bf16).ap()
    # mm2t: out stored transposed [octile, oc, tok]; host re-lays it out
    if mm2t:
        out_d = nc.dram_tensor("out", [2, _P, _T], f32, kind="ExternalOutput").ap()
    else:
        out_d = nc.dram_tensor("out", [_T, _OC], f32, kind="ExternalOutput").ap()

    def asf32(ap):
        return ap.bitcast(f32)

    import contextlib

    @contextlib.contextmanager
    def _nullpool():
        yield None

    with tile.TileContext(nc) as tc:
        with (
            tc.tile_pool(name="const", bufs=1) as const,
            tc.tile_pool(name="hs", bufs=2) as p_hs,
            tc.tile_pool(name="gb", bufs=2) as p_gb,
            tc.tile_pool(name="gsmall", bufs=2) as p_gs,
            tc.tile_pool(name="gaten", bufs=3) as p_gn,
            tc.tile_pool(name="ob", bufs=4) as p_ob,
            tc.tile_pool(name="psum_h", bufs=5 if ph5 else 4, space="PSUM") as psum_h,
            tc.tile_pool(name="psum_o", bufs=3 if po3 else 2, space="PSUM") as psum_o,
            tc.tile_pool(name="psum_g", bufs=1, space="PSUM") as psum_g,
            tc.tile_pool(name="psum_s", bufs=1, space="PSUM") if not (ph5 or po3) else _nullpool() as psum_s,
        ):
            xT_sb = const.tile([_P, 2, _T], xdt, name="xT_sb")
            w1_sb = const.tile([_P, 2, 4096], xdt, name="w1_sb")
            w2p_sb = const.tile([_P, _HM, _OC], bf16, name="w2p_sb")
            b1_sb = const.tile([_P, _HM], f32, name="b1_sb")
            wg_sb = const.tile([_P, 2, _E], xdt, name="wg_sb")
            bg_sb = const.tile([1, _E], xdt, name="bg_sb")
            bgc_sb = const.tile([_E, 1], f32, name="bgc_sb") if bgact else None
            b2w_sb = const.tile([_E, _OC], f32r, name="b2w_sb")
            ones_sb = const.tile([_E, _TN], xdt, name="ones_sb")

            for kc in range(2):
                nc.sync.dma_start(out=wg_sb[:, kc, :], in_=wg_d[kc])
            # x chunk 0 next: the gate matmul only needs wg + x0
            for kc in range(2):
                nc.sync.dma_start(out=xT_sb[:, kc, ts(0, _TN)], in_=xT_d[kc, 0])
            nc.sync.dma_start(out=bg_sb[:], in_=bg_d[:])
            if bgact:
                nc.sync.dma_start(out=bgc_sb[:], in_=bgc_d[:])
            nc.sync.dma_start(out=ones_sb[:], in_=ones_d[:])
            # b1 (16KB) before w1: the first relu drains gate on it; queued
            # behind w1's 2MB it stalls the chunk-0 psum_h drain ~2us
            nc.sync.dma_start(out=b1_sb[:], in_=b1_d[:])
            # w1 split fine (512KB pieces) so mm1 hm=0 starts early
            for q in range(4):
                for kc in range(2):
                    for hq in range(2):
                        nc.sync.dma_start(out=w1_sb[:, kc, ts(2 * q + hq, 512)],
                                          in_=w1_d[kc, q, :, ts(hq, 512)])
            # w2p/b2w before x1-7: mm2 of chunk 0 (interleaved into chunk 1's
            # mm1, ~t=30us) needs w2p; x chunk tn isn't needed until its own
            # mm1 window (~tn*27us), so it can queue behind the 2MB w2p.
            nc.sync.dma_start(out=b2w_sb[:], in_=b2w_d[:])
            nc.sync.dma_start(out=w2p_sb[:], in_=w2p_d[:])
            for tn in range(1, _NTN):
                for kc in range(2):
                    nc.sync.dma_start(out=xT_sb[:, kc, ts(tn, _TN)], in_=xT_d[kc, tn])

            def emit_gate(tn):
                pg = psum_g.tile([_E, _TN], f32, name="pg", tag="pg")
                nc.tensor.matmul(pg[:], wg_sb[:, 0, :], xT_sb[:, 0, ts(tn, _TN)],
                                 start=True, stop=False)
                if bgact:
                    nc.tensor.matmul(pg[:], wg_sb[:, 1, :], xT_sb[:, 1, ts(tn, _TN)],
                                     start=False, stop=True)
                else:
                    nc.tensor.matmul(pg[:], wg_sb[:, 1, :], xT_sb[:, 1, ts(tn, _TN)],
                                     start=False, stop=False)
                    nc.tensor.matmul(pg[:], bg_sb[:], ones_sb[0:1, :],
                                     start=False, stop=True)
                expu = p_gs.tile([_E, _TN], xdt, name="expu", tag="expu")
                if bgact:
                    nc.scalar.activation(expu[:], pg[:], AF.Exp, bias=bgc_sb[:, 0:1])
                else:
                    nc.scalar.activation(expu[:], pg[:], AF.Exp)
                if ph5 or po3:
                    ps = psum_g.tile([_E, _TN], f32, name="ps", tag="pg")[0:1, :]
                else:
                    ps = psum_s.tile([1, _TN], f32, name="ps", tag="ps")
                nc.tensor.matmul(ps[:], ones_sb[:, 0:1], expu[:],
                                 start=True, stop=True)
                rc = p_gs.tile([1, _TN], f32, name="rc", tag="rc")
                nc.vector.reciprocal(rc[:], ps[:])
                rcb = p_gs.tile([_E, _TN], f32, name="rcb", tag="rcb")
                nc.gpsimd.partition_broadcast(rcb[:], rc[:])
                gaten = p_gn.tile([_E, _TN], f32r, name="gaten", tag="gaten")
                nc.vector.tensor_mul(gaten[:], expu[:] if mm1bf else asf32(expu[:]),
                                     rcb[:])
                gatenb = p_gs.tile([_E, _TN], bf16, name="gatenb", tag="gatenb")
                nc.vector.tensor_copy(gatenb[:], asf32(gaten[:]))
                # gpsimd partition_broadcast needs base partition 0: DMA the 8
                # gate rows onto partition 0 of gb, then broadcast in place.
                gb = p_gb.tile([_P, _E, _TN], bf16, name="gb", tag="gb")
                if gbdma:
                    nc.sync.dma_start(out=gst_d[tn], in_=gatenb[:])
                    for e in range(_E):
                        src_bc = gst_d[tn, e:e + 1, :].broadcast_to((_P, _TN))
                        nc.sync.dma_start(out=gb[:, e, :], in_=src_bc)
                else:
                    nc.sync.dma_start(out=gb[0:1, :, :], in_=gatenb[:])
                    for e in range(_E):
                        nc.gpsimd.partition_broadcast(gb[:, e, :], gb[0:1, e, :])
                return gaten, gb

            def emit_mm1_pair(tn, hm, hs, gb):
                ph = psum_h.tile([_P, _TN], f32, name="ph", tag="ph")
                nc.tensor.matmul(ph[:], w1_sb[:, 0, ts(hm, _P)],
                                 xT_sb[:, 0, ts(tn, _TN)], start=True, stop=False)
                nc.tensor.matmul(ph[:], w1_sb[:, 1, ts(hm, _P)],
                                 xT_sb[:, 1, ts(tn, _TN)], start=False, stop=True)
                # relu(+bias) straight into bf16 hs, then scale by gate in place.
                # tn==0 has no previous-chunk mm2 to interleave, so the PE is
                # paced by the psum drain there: split that drain ACT/DVE (the
                # DVE is otherwise idle during tn==0's mm1).
                if not allact and tn0split and tn == 0 and hm % 2 == 1:
                    nc.vector.tensor_scalar(hs[:, hm, :], ph[:],
                                            b1_sb[:, hm:hm + 1], 0.0,
                                            mybir.AluOpType.add,
                                            mybir.AluOpType.max)
                elif not allact and dverelu and hm % 4 == 3:
                    nc.vector.tensor_scalar(hs[:, hm, :], ph[:],
                                            b1_sb[:, hm:hm + 1], 0.0,
                                            mybir.AluOpType.add,
                                            mybir.AluOpType.max)
                else:
                    nc.scalar.activation(hs[:, hm, :], ph[:], AF.Relu,
                                         bias=b1_sb[:, hm:hm + 1])
                if not nogmul:
                    eng = nc.gpsimd if (hm % 4) < poolmul else nc.vector
                    eng.tensor_mul(hs[:, hm, :], hs[:, hm, :], gb[:, hm // 4, :])

            po_open = {}

            def emit_mm2_half(tn, hs, gaten, tw, half):
                if half == 0:
                    po_open[tw] = psum_o.tile([_P, _OC], f32, name="po", tag="po")
                    for kt in range(_HM // 2):
                        nc.tensor.matmul(po_open[tw][:], hs[:, kt, ts(tw, _P)],
                                         w2p_sb[:, kt, :], start=(kt == 0), stop=False)
                    return
                po = po_open.pop(tw)
                for kt in range(_HM // 2, _HM):
                    nc.tensor.matmul(po[:], hs[:, kt, ts(tw, _P)], w2p_sb[:, kt, :],
                                     start=False, stop=False)
                nc.tensor.matmul(po[:], gaten[:, ts(tw, _P)], b2w_sb[:],
                                 start=False, stop=True)
                finish_mm2(tn, po, tw)

            def emit_mm2_quarter(tn, hs, gaten, tw, q):
                if q == 0:
                    po_open[tw] = psum_o.tile([_P, _OC], f32, name="po", tag="po")
                po = po_open[tw]
                for kt in range(q * (_HM // 4), (q + 1) * (_HM // 4)):
                    nc.tensor.matmul(po[:], hs[:, kt, ts(tw, _P)], w2p_sb[:, kt, :],
                                     start=(kt == 0), stop=False)
                if q == 3:
                    nc.tensor.matmul(po[:], gaten[:, ts(tw, _P)], b2w_sb[:],
                                     start=False, stop=True)
                    finish_mm2(tn, po_open.pop(tw), tw)

            def emit_mm2_block(tn, hs, gaten, tw):
                po = psum_o.tile([_P, _OC], f32, name="po", tag="po")
                for kt in range(_HM):
                    nc.tensor.matmul(po[:], hs[:, kt, ts(tw, _P)], w2p_sb[:, kt, :],
                                     start=(kt == 0), stop=False)
                nc.tensor.matmul(po[:], gaten[:, ts(tw, _P)], b2w_sb[:],
                                 start=False, stop=True)
                finish_mm2(tn, po, tw)

            def finish_mm2(tn, po, tw):
                row = (tn * (_TN // _P) + tw) * _P
                if podma:
                    nc.sync.dma_start(out=out_d[row:row + _P, :], in_=po[:])
                    return
                ob = p_ob.tile([_P, _OC], f32, name="ob", tag="ob")
                if obact:
                    nc.scalar.copy(ob[:], po[:])
                else:
                    nc.vector.tensor_copy(ob[:], po[:])
                nc.sync.dma_start(out=out_d[row:row + _P, :], in_=ob[:])

            NTW = _TN // _P  # mm2 token windows per chunk (4)
            HM_PER_TW = _HM // NTW  # mm1 pairs between mm2 blocks (8)

            def finish_mm2t(tn, po, octile):
                ob = p_ob.tile([_P, _TN], f32, name="ob", tag="ob")
                nc.scalar.copy(ob[:], po[:])
                nc.sync.dma_start(out=out_d[octile, :, ts(tn, _TN)], in_=ob[:])

            # mm2t quarter-chains for the fine interleave: octile chain of 32
            # kt + b2w split into 4 pieces, psum group held open across them
            po_t_open = {}

            def emit_mm2t_quarter(tn, hs, gaten, octile, q):
                if q == 0:
                    po_t_open[octile] = psum_o.tile([_P, _TN], f32, name="po",
                                                    tag="po")
                po = po_t_open[octile]
                for kt in range(q * (_HM // 4), (q + 1) * (_HM // 4)):
                    nc.tensor.matmul(po[:], w2p_sb[:, kt, ts(octile, _P)],
                                     hs[:, kt, :], start=(kt == 0), stop=False)
                if q == 3:
                    nc.tensor.matmul(po[:], b2w_sb[:, ts(octile, _P)], gaten[:],
                                     start=False, stop=True)
                    finish_mm2t(tn, po_t_open.pop(octile), octile)

            def mm2_steps(tn, hs, gaten):
                """Yield single mm2 PE ops for one chunk."""
                if mm2t:
                    # transposed: out[oc, tok] chains over kt with w2p
                    # stationary -> 2 chains of 33 N=512 matmuls per chunk
                    # (half the PE instructions of the tw-major form)
                    for octile in range(2):
                        po = psum_o.tile([_P, _TN], f32, name="po", tag="po")
                        for kt in range(_HM):
                            nc.tensor.matmul(po[:], w2p_sb[:, kt, ts(octile, _P)],
                                             hs[:, kt, :],
                                             start=(kt == 0), stop=False)
                            yield
                        nc.tensor.matmul(po[:], b2w_sb[:, ts(octile, _P)], gaten[:],
                                         start=False, stop=True)
                        finish_mm2t(tn, po, octile)
                        yield
                    return
                for tw in range(NTW):
                    po = psum_o.tile([_P, _OC], f32, name="po", tag="po")
                    for kt in range(_HM):
                        nc.tensor.matmul(po[:], hs[:, kt, ts(tw, _P)],
                                         w2p_sb[:, kt, :],
                                         start=(kt == 0), stop=False)
                        yield
                    nc.tensor.matmul(po[:], gaten[:, ts(tw, _P)], b2w_sb[:],
                                     start=False, stop=True)
                    finish_mm2(tn, po, tw)
                    yield

            def emit_body():
                pending = None
                for tn in range(_NTN):
                    gaten, gb = emit_gate(tn)
                    hs = p_hs.tile([_P, _HM, _TN], bf16, name="hs", tag="hs")
                    for hm in range(_HM):
                        emit_mm1_pair(tn, hm, hs, gb)
                        # interleave previous chunk's mm2 into the PE stream so
                        # the PE never stalls on the ACT-gated psum_h drain
                        if pending is None:
                            continue
                        if pp:
                            if mm2t:
                                n = ppk if ppk else 2  # 66 steps / 32 hm
                            else:
                                n = ppk if ppk else (5 if hm % 2 else 4)  # 136 / 32
                            for _ in range(n):
                                if next(pending[0], None) is None:
                                    break
                        elif fine == "q":
                            if hm % 2 == 1:
                                idx = hm // 2
                                emit_mm2_quarter(*pending[1], tw=idx // 4, q=idx % 4)
                        elif fine:
                            if hm % 4 == 3:
                                if mm2t:
                                    idx = hm // 4
                                    emit_mm2t_quarter(*pending[1],
                                                      octile=idx // 4, q=idx % 4)
                                else:
                                    emit_mm2_half(*pending[1], tw=hm // 8, half=(hm // 4) % 2)
                        elif hm % HM_PER_TW == HM_PER_TW - 1:
                            emit_mm2_block(*pending[1], tw=hm // HM_PER_TW)
                    if pending is not None and pp:
                        for _ in pending[0]:
                            pass
                    pending = (mm2_steps(tn, hs, gaten), (tn, hs, gaten))
                if pp:
                    for _ in pending[0]:
                        pass
                elif mm2t:
                    for octile in range(2):
                        for q in range(4):
                            emit_mm2t_quarter(*pending[1], octile=octile, q=q)
                else:
                    for tw in range(NTW):
                        emit_mm2_block(*pending[1], tw=tw)

            if loop > 1:
                with tc.For_i(0, loop, 1, staggered_reset=stagger):
                    emit_body()
            else:
                for _rep in range(reps):
                    emit_body()

    nc.compile()
    return nc


_MM1BF = True


def _xcast(a):
    import ml_dtypes
    if _MM1BF:
        return np.asarray(a, np.float32).astype(ml_dtypes.bfloat16)
    return _round_fp32r(a)


def _prep_weights(w1, b1, w2, b2, wg, bg, wo, bo):
    import ml_dtypes
    f32 = np.float32
    w1 = np.asarray(w1, f32)
    w2 = np.asarray(w2, f32)
    wo = np.asarray(wo, f32)
    E, IN, HID = w1.shape
    w1s = _xcast(np.ascontiguousarray(
        w1.transpose(1, 0, 2).reshape(IN, E * HID).reshape(2, _P, 4, 1024)
        .transpose(0, 2, 1, 3)))
    w2p = (w2.astype(np.float64).reshape(E * HID, -1) @ wo.astype(np.float64)).astype(f32)
    w2ps = np.ascontiguousarray(
        w2p.reshape(_HM, _P, _OC).transpose(1, 0, 2)).astype(ml_dtypes.bfloat16)
    b1s = np.ascontiguousarray(np.asarray(b1, f32).reshape(E * HID).reshape(_HM, _P).T)
    b2ws = _round_fp32r((np.asarray(b2, np.float64) @ wo.astype(np.float64)
                         + np.asarray(bo, np.float64)).astype(f32))
    wgs = _xcast(np.ascontiguousarray(np.asarray(wg, f32).reshape(2, _P, E)))
    bgs = _xcast(np.asarray(bg, f32).reshape(1, E))
    ones = np.ones((_E, _TN), f32)
    if _MM1BF:
        ones = ones.astype(ml_dtypes.bfloat16)
    bgc = np.ascontiguousarray(np.asarray(bg, f32).reshape(_E, 1))
    return dict(w1s=w1s, w2ps=w2ps, b1s=b1s, b2ws=b2ws, wgs=wgs, bgs=bgs,
                bgc=bgc, ones=ones)


def make_in_maps(x, w1, b1, w2, b2, wg, bg, wo, bo):
    x = np.asarray(x, np.float32)
    b, n, c = x.shape
    weights = _prep_weights(w1, b1, w2, b2, wg, bg, wo, bo)
    x2d = x.reshape(b * n, c)
    in_maps = []
    for i in range(_NCORES):
        xc = x2d[i * _T:(i + 1) * _T]                       # [T, C]
        xT = _xcast(np.ascontiguousarray(
            xc.T.reshape(2, _P, _NTN, _TN).transpose(0, 2, 1, 3)))
        in_maps.append({"xT": xT, **weights})
    return in_maps


def _run(x, w1, b1, w2, b2, wg, bg, wo, bo, trace=False):
    from concourse.bass_utils import run_bass_kernel_spmd

    if "nc" not in _CACHE:
        _CACHE["nc"] = _build_nc(1)
    nc = _CACHE["nc"]

    x = np.asarray(x, np.float32)
    b, n, c = x.shape
    in_maps = make_in_maps(x, w1, b1, w2, b2, wg, bg, wo, bo)

    res = run_bass_kernel_spmd(nc, in_maps, list(range(_NCORES)), trace=trace)
    def unshard(o):
        if o.ndim == 3:  # mm2t: [octile, oc, tok] -> [tok, oc]
            return o.transpose(2, 0, 1).reshape(_T, _OC)
        return o
    out = np.concatenate(
        [unshard(res.results[i]["out"]) for i in range(_NCORES)], axis=0)
    return out.reshape(b, n, _OC), res


def kernel(x, w1, b1, w2, b2, wg, bg, wo, bo):
    out, _ = _run(x, w1, b1, w2, b2, wg, bg, wo, bo, trace=False)
    return out

